# revision 21
# baseline (speedup 1.0000x reference)
"""Trainium2 Bass kernel for MultiHeadLatentAttention (B=2, T=2048, C=2048,
H=16, DH=128, L=512, interleaved RoPE on half of each head, causal SDPA).

Sharding: tensor-parallel over heads across 8 NeuronCores (2 heads/core).
Host collapses latent down+up projections into direct per-head weights
(W_up @ W_down), pre-transposes all weights into lhsT layout, precomputes
RoPE cos/sin tables, and sums the 8 row-parallel o_proj partial outputs.

Self-contained: only imports installed packages (numpy, ml_dtypes, concourse).
"""
import math
import numpy as np
import ml_dtypes

import concourse.bass as bass
import concourse.tile as tile
from concourse import bacc, mybir
from concourse.masks import make_identity
import concourse.bacc as _bacc_mod
import concourse.hw_specs as _hw_specs

# Pin every activation to the one table set containing exp+ln+copy so the
# Exp/Ln alternation in softmax normalization doesn't thrash ACT_TABLE_LOADs.
_ORIG_GAT = _hw_specs.get_activation_tables

def _pinned_tables(arch):
    t = _ORIG_GAT(arch)
    keep = "natural_log_exp_and_others"
    if keep not in t:
        return t
    return {k: (v if k == keep else set()) for k, v in t.items()}

_bacc_mod.get_activation_tables = _pinned_tables

# ---- problem constants ----
H = 16
C = 2048
DH = C // H          # 128
L = C // 4           # 512
THETA = 10000.0
N_CORES = 8
HPC = H // N_CORES   # heads per core = 2
P = 128
BF = mybir.dt.bfloat16
F32 = mybir.dt.float32

_BUILD_CACHE = {}


def build_nc(B, T, debug_taps=False, s_bufs=3, pvs_bufs=3):
    """Build the per-core Bass program (same program on all 8 cores)."""
    NT = B * T                 # total rows (b-major)
    KC = C // P                # 16 contraction chunks
    NCH = NT // 512            # 512-col n-chunks in phase 1
    NCHB = T // 512            # n-chunks per b
    TCH = T // P               # 128-wide t-chunks per b (16)
    QB = T // 512              # q-blocks per (b,h) (4)
    MW = 6 * P                 # stacked projection output width (768)

    nc = bacc.Bacc("TRN2", target_bir_lowering=False, debug=False,
                   enable_asserts=True, num_devices=N_CORES)

    xT_d = nc.dram_tensor("xT", [C, NT], BF, kind="ExternalInput").ap()
    Wp_d = nc.dram_tensor("Wp", [C, MW], BF, kind="ExternalInput").ap()
    cs_d = nc.dram_tensor("cs", [P, NT], BF, kind="ExternalInput").ap()
    sn_d = nc.dram_tensor("sn", [P, NT], BF, kind="ExternalInput").ap()
    WoT_d = nc.dram_tensor("WoT", [HPC * DH, C], BF, kind="ExternalInput").ap()
    tri_d = nc.dram_tensor("tri", [P, P], BF, kind="ExternalInput").ap()
    out_d = nc.dram_tensor("out_p", [NT, C], BF, kind="ExternalOutput").ap()
    if debug_taps:
        dbg_kT = nc.dram_tensor("dbg_kT", [P, HPC, NT], BF, kind="ExternalOutput").ap()
        dbg_qT = nc.dram_tensor("dbg_qT", [P, HPC, NT], BF, kind="ExternalOutput").ap()
        dbg_vS = nc.dram_tensor("dbg_vS", [P, HPC, NT // P, P], BF, kind="ExternalOutput").ap()
        dbg_aT = nc.dram_tensor("dbg_aT", [P, HPC, NT], BF, kind="ExternalOutput").ap()

    with tile.TileContext(nc) as tc:
        with (
            tc.tile_pool(name="const", bufs=1) as constp,
            tc.tile_pool(name="xt", bufs=3) as xtp,
            tc.tile_pool(name="kqv", bufs=1) as kqvp,
            tc.tile_pool(name="rtmp", bufs=4) as rtmp,
            tc.tile_pool(name="ptile", bufs=4) as ptp,
            tc.tile_pool(name="lnp", bufs=4) as lnp,
            tc.tile_pool(name="outp", bufs=3) as outp,
            tc.tile_pool(name="vtr", bufs=3) as vtrp,
            tc.tile_pool(name="ps_proj", bufs=2, space="PSUM") as psproj,
            tc.tile_pool(name="ps_s", bufs=s_bufs, space="PSUM") as pss,
            tc.tile_pool(name="ps_pvs", bufs=pvs_bufs, space="PSUM") as pspvs,
        ):
            pstr = psproj  # v-transpose psum piggybacks on the proj slots
            # ---- resident tensors (Wp per-chunk so k=0 arrives first) ----
            Wp_sb = constp.tile([P, KC, MW], BF, tag="Wp")
            Wp_r = Wp_d.rearrange("(ko p) m -> p ko m", p=P)
            for k in range(KC):
                nc.gpsimd.dma_start(Wp_sb[:, k, :], Wp_r[:, k, :])
            tri_sb = constp.tile([P, P], BF, tag="tri")
            nc.gpsimd.dma_start(tri_sb[:], tri_d)
            cs_sb = constp.tile([64, NT], BF, tag="cs")
            nc.gpsimd.dma_start(cs_sb[:], cs_d[0:64, :])
            sn_sb = constp.tile([64, NT], BF, tag="sn")
            nc.gpsimd.dma_start(sn_sb[:], sn_d[0:64, :])
            WoT_sb = constp.tile([P, HPC, C], BF, tag="WoT")
            nc.gpsimd.dma_start(WoT_sb[:], WoT_d.rearrange("(h p) c -> p h c", p=P))
            ones_sb = constp.tile([P, P], BF, tag="ones")
            nc.gpsimd.memset(ones_sb[:], 1.0)
            ident = constp.tile([P, P], BF, tag="ident")
            make_identity(nc, ident[:])

            kT = kqvp.tile([P, HPC, NT], BF, tag="kT")    # [dh, head, (b,t)]
            qT = kqvp.tile([P, HPC, NT], BF, tag="qT")
            vT = kqvp.tile([P, HPC, NT], BF, tag="vT")    # [dh, head, (b,t)]
            vS = kqvp.tile([P, HPC, NT // P, P], BF, tag="vS")  # [t%128, head, tchunk, dh]
            aT = kqvp.tile([P, HPC, NT], BF, tag="aT")    # attn^T [dh, head, (b,t)]

            def phase1(b):
                """Projection matmuls + epilogues for b's n-chunks."""
                for nch in range(NCHB):
                    n0 = b * T + nch * 512  # global col offset
                    xt = xtp.tile([P, KC, 512], BF, tag="xt")
                    xT_r = xT_d.rearrange("(ko p) n -> p ko n", p=P)
                    for k in range(KC):
                        nc.sync.dma_start(xt[:, k, :], xT_r[:, k, n0:n0 + 512])
                    cols = bass.ds(n0, 512)
                    for m in (1, 5, 0, 4, 2, 3):
                        ps_m = psproj.tile([P, 512], F32, tag="proj")
                        for k in range(KC):
                            nc.tensor.matmul(ps_m[:], Wp_sb[:, k, m * P:(m + 1) * P],
                                             xt[:, k, :], start=(k == 0),
                                             stop=(k == KC - 1))
                        if m == 0:    # k_c -> kT rows 0:64 per head
                            nc.scalar.copy(kT[0:64, 0, cols], ps_m[0:64, :])
                            nc.scalar.copy(kT[0:64, 1, cols], ps_m[64:128, :])
                        elif m == 4:  # q_c -> qT rows 0:64 per head
                            nc.scalar.copy(qT[0:64, 0, cols], ps_m[0:64, :])
                            nc.scalar.copy(qT[0:64, 1, cols], ps_m[64:128, :])
                        elif m in (2, 3):  # v^T chunks (head = m - 2)
                            nc.scalar.copy(vT[:, m - 2, cols], ps_m[:])
                        else:         # rope linear for k (m=1) or q (m=5)
                            dst = kT if m == 1 else qT
                            E, O = ps_m[0:64, :], ps_m[64:128, :]
                            csx, snx = cs_sb[0:64, cols], sn_sb[0:64, cols]
                            # both-SBUF TT inputs must share a partition base,
                            # so pair each combine's operands inside one tile
                            mt_e = rtmp.tile([64, 2, 512], BF, tag="rte")
                            mt_o = rtmp.tile([64, 2, 512], BF, tag="rto")
                            TT = nc.vector.tensor_tensor
                            TT(mt_e[:, 0, :], E, csx, mybir.AluOpType.mult)
                            TT(mt_e[:, 1, :], O, snx, mybir.AluOpType.mult)
                            TT(mt_o[:, 0, :], O, csx, mybir.AluOpType.mult)
                            TT(mt_o[:, 1, :], E, snx, mybir.AluOpType.mult)
                            for h in range(HPC):
                                hs = slice(32 * h, 32 * h + 32)
                                TT(dst[64:96, h, cols], mt_e[hs, 0, :], mt_e[hs, 1, :],
                                   mybir.AluOpType.subtract)
                                TT(dst[96:128, h, cols], mt_o[hs, 0, :], mt_o[hs, 1, :],
                                   mybir.AluOpType.add)
                    # transpose this n-chunk's v slices right away
                    for h in range(HPC):
                        for tc_i in range(4):
                            g = (n0 // P) + tc_i
                            tr_ps = pstr.tile([P, P], BF, tag="proj")
                            nc.tensor.transpose(tr_ps[:], vT[:, h, g * P:(g + 1) * P],
                                                ident[:])
                            nc.vector.tensor_copy(vS[:, h, g, :], tr_ps[:])

            def attention(b, h, Q_range):
                kTh = kT[:, h, :]
                qTh = qT[:, h, :]
                for Q in Q_range:
                    q0 = Q * 512            # local col offset within b
                    jmax = 4 * Q + 3
                    pv = pspvs.tile([P, 512], F32, tag="pvs")
                    sums = pspvs.tile([P, 512], F32, tag="pvs")
                    pending = []
                    for j in range(jmax + 1):
                        cstart = max(q0, j * P)
                        n = q0 + 512 - cstart
                        gcol = bass.ds(b * T + cstart, n)
                        s_ps = pss.tile([P, 512], F32, tag="s")
                        nc.tensor.matmul(s_ps[:, 0:n], kTh[:, b * T + j * P:
                                                           b * T + (j + 1) * P],
                                         qTh[:, gcol], start=True, stop=True)
                        p = ptp.tile([P, 512], BF, tag="p")
                        nc.scalar.activation(p[:, 0:n], s_ps[:, 0:n],
                                             mybir.ActivationFunctionType.Exp)
                        if j * P >= q0:  # diagonal block: mask first 128 cols
                            nc.vector.tensor_tensor(p[:, 0:P], p[:, 0:P], tri_sb[:],
                                                    mybir.AluOpType.mult)
                        pending.append((j, jmax, q0, p, n, cstart))
                        if len(pending) > 2:
                            _emit_pv(b, h, pv, sums, *pending.pop(0))
                    for pe in pending:
                        _emit_pv(b, h, pv, sums, *pe)
                    # normalize: attn = pv * exp(-ln(sums))
                    ln_t = lnp.tile([P, 512], F32, tag="ln")
                    nc.scalar.activation(ln_t[:], sums[:],
                                         mybir.ActivationFunctionType.Ln)
                    rec = lnp.tile([P, 512], F32, tag="ln")
                    nc.scalar.activation(rec[:], ln_t[:],
                                         mybir.ActivationFunctionType.Exp,
                                         scale=-1.0)
                    nc.vector.tensor_tensor(aT[:, h, bass.ds(b * T + q0, 512)],
                                            pv[:], rec[:], mybir.AluOpType.mult)

            def _emit_pv(b, h, pv, sums, j, jmax, q0, p, n, cstart):
                off = cstart - q0
                nc.tensor.matmul(pv[:, off:off + n], vS[:, h, b * TCH + j, :],
                                 p[:, 0:n], start=(j == 0), stop=(j == jmax))
                nc.tensor.matmul(sums[:, off:off + n], ones_sb[:], p[:, 0:n],
                                 start=(j == 0), stop=(j == jmax))

            def emit_o_unit(b, tt, cc, pool=None, ptag="proj"):
                r0 = b * T + tt * P
                o_ps = (pool or psproj).tile([P, 512], F32, tag=ptag)
                for h in range(HPC):
                    nc.tensor.matmul(o_ps[:],
                                     aT[:, h, r0:r0 + P],
                                     WoT_sb[:, h, cc * 512:(cc + 1) * 512],
                                     start=(h == 0), stop=(h == HPC - 1))
                o_sb = outp.tile([P, 512], BF, tag="osb")
                nc.vector.tensor_copy(o_sb[:], o_ps[:])
                nc.sync.dma_start(out_d[r0:r0 + P, cc * 512:(cc + 1) * 512], o_sb[:])

            def o_units(b):
                return [(b, tt, cc) for tt in range(TCH) for cc in range(C // 512)]

            prev_units = []
            for b in range(B):
                phase1(b)
                ui = 0
                for Q in range(QB):
                    for h in range(HPC):
                        attention(b, h, [Q])
                        # drip previous b's o_proj into this attention phase
                        tgt = len(prev_units) * (Q * HPC + h + 1) // (QB * HPC)
                        while ui < tgt:
                            emit_o_unit(*prev_units[ui]); ui += 1
                while ui < len(prev_units):
                    emit_o_unit(*prev_units[ui]); ui += 1
                prev_units = o_units(b)
            pools = [(psproj, "proj"), (pss, "s"), (pspvs, "pvs")]
            for i, u in enumerate(prev_units):
                pl, tg = pools[i % 3]
                emit_o_unit(*u, pool=pl, ptag=tg)
            if debug_taps:
                nc.sync.dma_start(dbg_kT, kT[:])
                nc.sync.dma_start(dbg_qT, qT[:])
                nc.sync.dma_start(dbg_vS, vS[:])
                nc.sync.dma_start(dbg_aT, aT[:])

    nc.finalize()
    from concourse.bass_interp import get_hw_module
    nc.m = get_hw_module(nc.m)
    return nc


def _bf(a):
    return np.ascontiguousarray(a).astype(ml_dtypes.bfloat16)


def make_in_maps(x, W_kvD, W_qD, W_kU, W_vU, W_qU, W_rk, W_rq, W_o):
    """Host-side input marshaling: collapse latents, slice per head, transpose."""
    B, T, _ = x.shape
    NT = B * T
    scale = 1.0 / math.sqrt(DH)

    xT = x.transpose(2, 0, 1).reshape(C, NT)          # [C, (b,t)] b-major
    W_kc = W_kU @ W_kvD                               # [C/2, C]
    W_v = W_vU @ W_kvD                                # [C, C]
    W_qc = (W_qU @ W_qD) * scale                      # [C/2, C]
    W_qr = (W_rq @ W_qD) * scale                      # [C/2, C]
    W_kr = W_rk                                       # [C/2, C]

    # rope tables over pair index p: freq_p = THETA ** (-p / (C/4))
    pos = np.arange(T, dtype=np.float64)
    pairs = np.arange(C // 4, dtype=np.float64)       # 512 pairs over D=1024
    freqs = THETA ** (-pairs / (C // 4))
    ang = pos[None, :] * freqs[:, None]               # [512, T]
    cos_full = np.cos(ang)
    sin_full = np.sin(ang)

    tri = np.triu(np.ones((P, P), dtype=np.float32))

    in_maps = []
    for c in range(N_CORES):
        hd0 = c * HPC * DH                            # first head dim offset
        plist = np.arange(64 * c, 64 * c + 64)        # rope pairs (h0 then h1)
        rperm = np.concatenate([2 * plist, 2 * plist + 1])  # evens then odds
        Wp = np.concatenate([
            W_kc[c * P:(c + 1) * P].T,                # k content   [C,128]
            W_kr[rperm].T,                            # k rope      [C,128]
            W_v[hd0:hd0 + HPC * DH].T,                # v           [C,256]
            W_qc[c * P:(c + 1) * P].T,                # q content   [C,128]
            W_qr[rperm].T,                            # q rope      [C,128]
        ], axis=1)                                    # [C, 768]
        cs64 = np.tile(cos_full[plist], (1, B))       # [64, NT] b-major
        sn64 = np.tile(sin_full[plist], (1, B))
        cs = np.concatenate([cs64, cs64], axis=0)     # duplicate for odd rows
        sn = np.concatenate([sn64, sn64], axis=0)
        in_maps.append({
            "xT": _bf(xT),
            "Wp": _bf(Wp),
            "cs": _bf(cs),
            "sn": _bf(sn),
            "WoT": _bf(W_o[:, hd0:hd0 + HPC * DH].T),
            "tri": _bf(tri),
        })
    return in_maps


def kernel(x, W_kvD, W_qD, W_kU, W_vU, W_qU, W_rk, W_rq, W_o):
    B, T, _ = x.shape
    key = (B, T)
    if key not in _BUILD_CACHE:
        _BUILD_CACHE[key] = build_nc(B, T)
    nc = _BUILD_CACHE[key]
    in_maps = make_in_maps(x, W_kvD, W_qD, W_kU, W_vU, W_qU, W_rk, W_rq, W_o)
    from concourse.bass_utils import run_bass_kernel_spmd
    res = run_bass_kernel_spmd(nc, in_maps, core_ids=list(range(N_CORES)))
    out = np.zeros((B * T, C), dtype=np.float32)
    for c in range(N_CORES):
        out += res.results[c]["out_p"].astype(np.float32)
    return out.reshape(B, T, C)
